# revision 33
# baseline (speedup 1.0000x reference)
"""Additive-attention kernel for 8 TRN2 NeuronCores.

reference:
    x = concat([s, h], axis=1)            # (N, 2D)
    X = tanh(x @ W.T)                     # (N, 2*DA)
    pre = (X @ v.T).T                     # (1, N)
    out = softmax(pre, axis=1)            # (1, N)

Strategy: shard rows (N) across 8 cores (4096 rows each). W, v replicated.
Each core computes tanh(x_shard @ W.T) @ v.T fused in SBUF/PSUM (bf16
matmul, fp32 accumulate), produces 4096 scores, takes exp, sums locally,
AllReduces the per-core sums, and normalizes its shard by the global sum.
Softmax max-subtraction is skipped: |score| <= ||v||_1 ~ 33 << 88 (fp32 exp
overflow), so exp is always finite and the result is exact to fp32.

The matmul phase runs at the power-throttled PE roofline: under
sustained 8-core load the PE is clock-limited to ~1.95-2.0 GHz (either
the board GPIO throttler gating to K=13/16 of 2.4 GHz, or the P0 power
state), giving ~263 ns per 128x128x512 bf16 matmul; measured PE idle
within the span is <2%. fp8 DoubleRow (2x PE rate, verified 216 ns for
K=256 on this HW) was evaluated and rejected: e4m3 quantization of both
operands yields 3.1e-2 final error (vs the 2e-2 gate, measured against
the seeded reference), and any residual-corrected scheme costs as many
PE cycles as bf16. The structure minimizes everything around the PE:
 - a short warm-up matmul run on the first arrived x slice keeps the
   HAM activity clock-gate open through the DMA-bound head.
 - W streams as full k-tiles in consumption order; x prefetch and the
   v replica are issued behind it (W for all 8 cores is 67 MB of HBM
   reads - the head is HBM-bound, so W owns the early window).
 - row-tiles 0 and 1 interleave over k so each arriving W k-tile feeds
   8 matmuls; later tiles run k-outer / j-inner (4 consecutive matmuls
   share the stationary x-tile; redundant LDWEIGHTS stripped
   post-build). The last tile is chunk-major so its drain pipelines.
 - tail: one Exp with fused accumulate, partition reduce, a single
   4-byte AllReduce(add) of the 8 partial sums, broadcast, scale,
   store. ~5 us from last matmul to collective trigger.

Host-side prep is layout only (transpose/concat/cast + replicate v).
"""

import numpy as np
import ml_dtypes

N, D, DA = 32768, 1024, 1024
NCORES = 8
NS = N // NCORES            # 4096 rows per core
P = 128
MT = NS // P                # 32 row-tiles per core
KIN = 2 * D                 # 2048 contraction
KT = KIN // P               # 16 k-tiles
NOUT = 2 * DA               # 2048 out features
NCH = 512                   # psum chunk (one bank of fp32)
NCK = NOUT // NCH           # 4 chunks


def _build_nc():
    from concourse import bacc, mybir, tile, bass

    f32 = mybir.dt.float32
    bf16 = mybir.dt.bfloat16
    AF = mybir.ActivationFunctionType
    ALU = mybir.AluOpType
    AX = mybir.AxisListType

    nc = bacc.Bacc(
        "TRN2",
        target_bir_lowering=False,
        debug=False,
        num_devices=NCORES,
    )

    xh = nc.declare_dram_parameter("xh", [NS, KIN], bf16, isOutput=False)
    wt = nc.declare_dram_parameter("wt", [KIN, NOUT], bf16, isOutput=False)
    vr = nc.declare_dram_parameter("vr", [P, NOUT], f32, isOutput=False)
    out_ext = nc.declare_dram_parameter("out", [P, MT], f32, isOutput=True)

    with tile.TileContext(nc) as tc:
        with (
            tc.tile_pool(name="wpool", bufs=1) as wpool,
            tc.tile_pool(name="xpool", bufs=4) as xpool,
            tc.tile_pool(name="tpool", bufs=3) as tpool,
            tc.tile_pool(name="spool", bufs=1) as spool,
            tc.tile_pool(name="ppool", bufs=2, space="PSUM") as ppool,
            tc.tile_pool(name="dpool", bufs=1, space="DRAM") as dpool,
        ):
            # first x k-slice, then W tiles in k (consumption) order.
            # W owns the DMA rings early: x prefetch and v are issued behind
            # the W descriptors so W tiles complete as early as possible.
            xm0 = xpool.tile([P, KIN], bf16, name="xm", tag="xm")
            nc.sync.dma_start(out=xm0[:, 0:P], in_=xh[0:P, 0:P])
            wsb = [
                wpool.tile([P, NOUT], bf16, name=f"wk{k}") for k in range(KT)
            ]
            # w0 in halves: the first real matmuls (k0, j0/j1) only wait on
            # the first 1024 columns, starting ~1.5us earlier.
            # ALL W rides the sync queue in strict k order (x/v go on
            # scalar): W rows then hit the rings in exactly consumption
            # order, instead of interleaving across two queues' ring
            # assignments and completing lumpily.
            nc.sync.dma_start(out=wsb[0][:, 0:1024], in_=wt[0:P, 0:1024])
            nc.scalar.dma_start(out=xm0[:, P:1024], in_=xh[0:P, P:1024])
            nc.sync.dma_start(out=wsb[0][:, 1024:NOUT], in_=wt[0:P, 1024:NOUT])
            nc.scalar.dma_start(out=xm0[:, 1024:KIN], in_=xh[0:P, 1024:KIN])
            for k in range(1, KT // 2):
                nc.sync.dma_start(
                    out=wsb[k][:, :], in_=wt[k * P:(k + 1) * P, :]
                )

            # prioritize W k0..7 on the DMA rings: the rings round-robin all
            # queued rows, so issuing all 16 tiles at once makes early
            # k-tiles complete as late as the last ones and stalls the
            # in-order PE queue. This tiny SBUF->DRAM dma stalls the sync
            # queue until k7 lands, so k8..15 only hit the rings afterwards
            # (consumption of k8..15 starts ~6us later than that).
            wh_gate = dpool.tile([1, 1], bf16, name="wh_gate")
            nc.sync.dma_start(out=wh_gate[:, :], in_=wsb[KT // 2 - 1][0:1, 0:1])
            for k in range(KT // 2, KT):
                nc.sync.dma_start(
                    out=wsb[k][:, :], in_=wt[k * P:(k + 1) * P, :]
                )

            # PE pre-warm on the first x slice (lands ~1.5us in): keeps the
            # PE busy so the HAM activity clock-gate opens before real work;
            # results land in a psum bank that the real stream later resets
            pswarm = ppool.tile([P, NCH], f32, name="ps0", tag="ps0")
            for _ in range(22):
                nc.tensor.matmul(
                    pswarm[:, 0:P], lhsT=xm0[:, 0:P], rhs=xm0[:, 0:P],
                    start=True, stop=True,
                )

            def load_xm(m, eng):
                t = xpool.tile([P, KIN], bf16, name="xm", tag="xm")
                eng.dma_start(out=t[:, :], in_=xh[m * P:(m + 1) * P, :])
                return t

            xm_pre = [xm0, load_xm(1, nc.scalar)]

            # rendezvous the 8 cores while the weight DMAs stream in, so the
            # tail collective doesn't pay launch-skew latency
            sync_in = dpool.tile([1, 1], f32, name="sync_in")
            sync_out = dpool.tile(
                [1, NCORES], f32, name="sync_out", addr_space="Shared"
            )
            nc.gpsimd.collective_compute(
                "AllGather",
                ALU.bypass,
                replica_groups=[list(range(NCORES))],
                ins=[sync_in.opt()],
                outs=[sync_out.opt()],
            )
            # v replica loads on scalar with the x traffic (needed ~45us in)
            vsb = wpool.tile([P, NOUT], f32, name="vsb")
            nc.scalar.dma_start(out=vsb[:, :], in_=vr[:, :])

            # gate the early x prefetches behind W completion: this copy
            # stalls the gpsimd queue until the last W tile lands, so the
            # prefetch DMAs it issues next can't steal ring bandwidth from
            # the W stream (their deadline is ~48us+)
            wgate = spool.tile([1, 1], bf16, name="wgate")
            nc.gpsimd.tensor_copy(wgate[0:1, 0:1], wsb[KT - 1][0:1, 0:1])

            scores = spool.tile([P, MT], f32, name="scores")
            expv = spool.tile([P, MT], f32, name="expv")
            zrow = spool.tile([P, 1], f32, name="zrow")

            def alloc_work(m):
                psums = []
                for j in range(NCK):
                    ps = ppool.tile([P, NCH], f32, name=f"ps{j}", tag=f"ps{j}")
                    psums.append(ps)
                tmt = tpool.tile([P, NOUT], f32, name="tmt", tag="tmt")
                umt = tpool.tile([P, NOUT], f32, name="umt", tag="umt")
                acc = tpool.tile([P, NCK], f32, name="acc", tag="acc")
                return psums, tmt, umt, acc

            def drain(m, psums, tmt, umt, acc, j):
                sl = slice(j * NCH, (j + 1) * NCH)
                nc.scalar.activation(tmt[:, sl], psums[j][:, :], AF.Tanh)
                # one DVE op: umt = tanh*v, acc[:,j] = row-sum(umt)
                nc.vector.scalar_tensor_tensor(
                    out=umt[:, sl],
                    in0=tmt[:, sl],
                    scalar=1.0,
                    in1=vsb[:, sl],
                    op0=ALU.mult,
                    op1=ALU.mult,
                    accum_out=acc[:, j:j + 1],
                )

            def finish_scores(m, acc):
                nc.vector.tensor_reduce(
                    scores[:, m:m + 1], acc[:, :], AX.X, ALU.add
                )

            # tiles 0 and 1 interleaved over k: 8 matmuls per arriving W
            # k-tile keep the PE saturated while W streams in (8.4 MB takes
            # ~25us; a single tile only holds 17us of work)
            work01 = [alloc_work(0), alloc_work(1)]
            for k in range(KT):
                for m in (0, 1):
                    for j in range(NCK):
                        nc.tensor.matmul(
                            work01[m][0][j][:, :],
                            lhsT=xm_pre[m][:, k * P:(k + 1) * P],
                            rhs=wsb[k][:, j * NCH:(j + 1) * NCH],
                            start=(k == 0),
                            stop=(k == KT - 1),
                        )
            for m in (0, 1):
                psums, tmt, umt, acc = work01[m]
                for j in range(NCK):
                    drain(m, psums, tmt, umt, acc, j)
                finish_scores(m, acc)

            for m in range(2, MT):
                # early prefetches go on the gpsimd queue behind the W-gate
                # copy; later ones are gated by xpool instance reuse anyway
                if m < 10:
                    eng = nc.gpsimd
                else:
                    eng = nc.sync if m % 2 == 0 else nc.gpsimd
                xm = load_xm(m, eng)
                psums, tmt, umt, acc = alloc_work(m)

                if m < MT - 1:
                    # k-outer: the 4 matmuls per k share the stationary x
                    # tile (LDWEIGHTS dedup below)
                    for k in range(KT):
                        for j in range(NCK):
                            nc.tensor.matmul(
                                psums[j][:, :],
                                lhsT=xm[:, k * P:(k + 1) * P],
                                rhs=wsb[k][:, j * NCH:(j + 1) * NCH],
                                start=(k == 0),
                                stop=(k == KT - 1),
                            )
                    for j in range(NCK):
                        drain(m, psums, tmt, umt, acc, j)
                    finish_scores(m, acc)
                else:
                    # last tile chunk-major so each chunk drains while the
                    # next chunk's matmuls run, and in half-chunks so the
                    # final tanh+mul on the critical chain is half as long:
                    # shortens every core's path to the collective trigger
                    acc8 = tpool.tile(
                        [P, 2 * NCK], f32, name="acc8", tag="acc8"
                    )
                    NH = NCH // 2
                    for j in range(NCK):
                        for k in range(KT):
                            nc.tensor.matmul(
                                psums[j][:, :],
                                lhsT=xm[:, k * P:(k + 1) * P],
                                rhs=wsb[k][:, j * NCH:(j + 1) * NCH],
                                start=(k == 0),
                                stop=(k == KT - 1),
                            )
                        for h in range(2):
                            sl = slice(
                                j * NCH + h * NH, j * NCH + (h + 1) * NH
                            )
                            psl = slice(h * NH, (h + 1) * NH)
                            nc.scalar.activation(
                                tmt[:, sl], psums[j][:, psl], AF.Tanh
                            )
                            nc.vector.scalar_tensor_tensor(
                                out=umt[:, sl],
                                in0=tmt[:, sl],
                                scalar=1.0,
                                in1=vsb[:, sl],
                                op0=ALU.mult,
                                op1=ALU.mult,
                                accum_out=acc8[:, 2 * j + h:2 * j + h + 1],
                            )
                    nc.vector.tensor_reduce(
                        scores[:, m:m + 1], acc8[:, :], AX.X, ALU.add
                    )

            # ---- softmax over the global N via one AllReduce ----
            nc.scalar.activation(
                expv[:, :], scores[:, :], AF.Exp, accum_out=zrow[:, 0:1]
            )
            zloc = spool.tile([1, 1], f32, name="zloc")
            nc.gpsimd.tensor_reduce(
                zloc[0:1, 0:1], zrow[:, 0:1], AX.C, ALU.add
            )
            # AllGather instead of AllReduce: the gather is a 2-phase mesh
            # op vs AllReduce's 4 phases (~2-3us cheaper after the last
            # contribution); the 8-way add costs one ~80ns DVE op instead
            zin = dpool.tile([1, 1], f32, name="zin")
            zout = dpool.tile(
                [1, NCORES], f32, name="zout", addr_space="Shared"
            )
            nc.gpsimd.dma_start(out=zin[:, :], in_=zloc[0:1, 0:1])
            nc.gpsimd.collective_compute(
                "AllGather",
                ALU.bypass,
                replica_groups=[list(range(NCORES))],
                ins=[zin.opt()],
                outs=[zout.opt()],
            )
            # broadcast the 8 partials to every partition (stride-0 DRAM
            # read), reduce + reciprocal once, then scale the shard, store
            zgb = spool.tile([P, NCORES], f32, name="zgb")
            zout_bc = bass.AP(
                zout.tensor, zout.offset, [(0, P), (1, NCORES)]
            )
            nc.gpsimd.dma_start(out=zgb[:, :], in_=zout_bc)
            zp = spool.tile([P, 1], f32, name="zp")
            nc.vector.tensor_reduce(zp[:, 0:1], zgb[:, :], AX.X, ALU.add)
            rzb = spool.tile([P, 1], f32, name="rzb")
            nc.vector.reciprocal(rzb[:, 0:1], zp[:, 0:1])
            outsb = spool.tile([P, MT], f32, name="outsb")
            nc.vector.tensor_scalar_mul(outsb[:, :], expv[:, :], rzb[:, 0:1])
            nc.sync.dma_start(out=out_ext[:, :], in_=outsb[:, :])

    # run_bass_via_pjrt binds the exec primitive directly and skips the
    # finalize that bass_jit flows do; Bacc register allocation runs here.
    nc.finalize()
    _strip_redundant_ldweights(nc)
    return nc


def _strip_redundant_ldweights(nc):
    """Bacc's move_matmul_waits_to_ldweights emits one InstLdweights per
    matmul even when consecutive matmuls share the stationary operand.
    The PE keeps the loaded weights across matmuls, so an Ldweights whose
    weights AP equals the previous one's and that carries no semaphore
    waits/updates is pure redundant load time (~110ns each on the PE
    critical path). Drop them; only the matmuls (ldweights=false) remain."""
    def sig(arg):
        return (
            getattr(arg, "memref", None),
            getattr(arg, "offset", None),
            str(getattr(arg, "ap", None)),
        )

    removed = 0
    for bb in nc.main_func.blocks:
        keep = []
        last = None
        for inst in bb.instructions:
            if "Ldweights" in type(inst).__name__:
                s = sig(inst.ins[0])
                si = inst.sync_info
                if s == last and (
                    si is None or (not si.on_wait and not si.on_update)
                ):
                    removed += 1
                    continue
                last = s
            keep.append(inst)
        bb.instructions = keep
    return removed


def _prep_core_inputs(s, h, W, v):
    """Host-side layout prep: per-core tiled x^T, shared W^T, replicated v."""
    bf16 = ml_dtypes.bfloat16
    wt = np.ascontiguousarray(W.T).astype(bf16)          # [KIN, NOUT]
    vrep = np.ascontiguousarray(
        np.broadcast_to(v.reshape(1, NOUT), (P, NOUT))
    ).astype(np.float32)

    in_maps = []
    for c in range(NCORES):
        sl = slice(c * NS, (c + 1) * NS)
        x = np.concatenate([s[sl], h[sl]], axis=1)       # [NS, KIN]
        # xh[m*128+kk, k*128+rr] = x[m*128+rr, k*128+kk]
        xh = (
            x.reshape(MT, P, KT, P)
            .transpose(0, 3, 2, 1)
            .reshape(NS, KIN)
        )
        xh = np.ascontiguousarray(xh).astype(bf16)
        in_maps.append({"xh": xh, "wt": wt, "vr": vrep})
    return in_maps


_RUN_KW = {}  # test.py can inject trace=True etc.
LAST_RESULT = None


def kernel(s, h, W, v):
    from concourse.bass_utils import run_bass_kernel_spmd

    global LAST_RESULT
    s = np.asarray(s, dtype=np.float32)
    h = np.asarray(h, dtype=np.float32)
    W = np.asarray(W, dtype=np.float32)
    v = np.asarray(v, dtype=np.float32)

    in_maps = _prep_core_inputs(s, h, W, v)
    res = None
    for attempt in range(3):
        nc = _build_nc()
        try:
            res = run_bass_kernel_spmd(
                nc, in_maps, core_ids=list(range(NCORES)), **_RUN_KW
            )
            break
        except Exception:
            # transient NRT_EXEC_UNIT_UNRECOVERABLE states clear on the
            # next attempt; rebuild and retry
            if attempt == 2:
                raise
            import time
            time.sleep(15)
    LAST_RESULT = res

    outs = []
    for c in range(NCORES):
        oc = np.asarray(res.results[c]["out"], dtype=np.float32)  # [P, MT]
        outs.append(oc.T.reshape(-1))                              # rows m*128+p
    return np.concatenate(outs).reshape(1, N).astype(np.float32)


# revision 35
# speedup vs baseline: 1.0052x; 1.0052x over previous
"""Additive-attention kernel for 8 TRN2 NeuronCores.

reference:
    x = concat([s, h], axis=1)            # (N, 2D)
    X = tanh(x @ W.T)                     # (N, 2*DA)
    pre = (X @ v.T).T                     # (1, N)
    out = softmax(pre, axis=1)            # (1, N)

Strategy: shard rows (N) across 8 cores (4096 rows each). W, v replicated.
Each core computes tanh(x_shard @ W.T) @ v.T fused in SBUF/PSUM (bf16
matmul, fp32 accumulate), produces 4096 scores, takes exp, sums locally,
AllReduces the per-core sums, and normalizes its shard by the global sum.
Softmax max-subtraction is skipped: |score| <= ||v||_1 ~ 33 << 88 (fp32 exp
overflow), so exp is always finite and the result is exact to fp32.

The matmul phase runs at the power-throttled PE roofline: under
sustained 8-core load the PE is clock-limited to ~1.95-2.0 GHz (either
the board GPIO throttler gating to K=13/16 of 2.4 GHz, or the P0 power
state), giving ~263 ns per 128x128x512 bf16 matmul; measured PE idle
within the span is <2%. fp8 DoubleRow (2x PE rate, verified 216 ns for
K=256 on this HW) was evaluated and rejected: e4m3 quantization of both
operands yields 3.1e-2 final error (vs the 2e-2 gate, measured against
the seeded reference), and any residual-corrected scheme costs as many
PE cycles as bf16. The structure minimizes everything around the PE:
 - a short warm-up matmul run on the first arrived x slice keeps the
   HAM activity clock-gate open through the DMA-bound head.
 - W streams as full k-tiles in consumption order; x prefetch and the
   v replica are issued behind it (W for all 8 cores is 67 MB of HBM
   reads - the head is HBM-bound, so W owns the early window).
 - row-tiles 0 and 1 interleave over k so each arriving W k-tile feeds
   8 matmuls; later tiles run k-outer / j-inner (4 consecutive matmuls
   share the stationary x-tile; redundant LDWEIGHTS stripped
   post-build). The last tile is chunk-major so its drain pipelines.
 - tail: one Exp with fused accumulate, partition reduce, a single
   4-byte AllReduce(add) of the 8 partial sums, broadcast, scale,
   store. ~5 us from last matmul to collective trigger.

Host-side prep is layout only (transpose/concat/cast + replicate v).
"""

import numpy as np
import ml_dtypes

N, D, DA = 32768, 1024, 1024
NCORES = 8
NS = N // NCORES            # 4096 rows per core
P = 128
MT = NS // P                # 32 row-tiles per core
KIN = 2 * D                 # 2048 contraction
KT = KIN // P               # 16 k-tiles
NOUT = 2 * DA               # 2048 out features
NCH = 512                   # psum chunk (one bank of fp32)
NCK = NOUT // NCH           # 4 chunks


def _build_nc():
    from concourse import bacc, mybir, tile, bass

    f32 = mybir.dt.float32
    bf16 = mybir.dt.bfloat16
    AF = mybir.ActivationFunctionType
    ALU = mybir.AluOpType
    AX = mybir.AxisListType

    nc = bacc.Bacc(
        "TRN2",
        target_bir_lowering=False,
        debug=False,
        num_devices=NCORES,
    )

    xh = nc.declare_dram_parameter("xh", [NS, KIN], bf16, isOutput=False)
    wt = nc.declare_dram_parameter("wt", [KIN, NOUT], bf16, isOutput=False)
    vr = nc.declare_dram_parameter("vr", [P, NOUT], f32, isOutput=False)
    out_ext = nc.declare_dram_parameter("out", [P, MT], f32, isOutput=True)

    with tile.TileContext(nc) as tc:
        with (
            tc.tile_pool(name="wpool", bufs=1) as wpool,
            tc.tile_pool(name="xpool", bufs=4) as xpool,
            tc.tile_pool(name="tpool", bufs=3) as tpool,
            tc.tile_pool(name="spool", bufs=1) as spool,
            tc.tile_pool(name="ppool", bufs=2, space="PSUM") as ppool,
            tc.tile_pool(name="dpool", bufs=1, space="DRAM") as dpool,
        ):
            # first x k-slice, then W tiles in k (consumption) order.
            # W owns the DMA rings early: x prefetch and v are issued behind
            # the W descriptors so W tiles complete as early as possible.
            xm0 = xpool.tile([P, KIN], bf16, name="xm", tag="xm")
            nc.sync.dma_start(out=xm0[:, 0:P], in_=xh[0:P, 0:P])
            wsb = [
                wpool.tile([P, NOUT], bf16, name=f"wk{k}") for k in range(KT)
            ]
            # w0 in pieces: the first real matmul (k0, j0) only waits on
            # the first 512 columns, starting ~1us earlier still.
            # ALL W rides the sync queue in strict k order (x/v go on
            # scalar): W rows then hit the rings in exactly consumption
            # order, instead of interleaving across two queues' ring
            # assignments and completing lumpily.
            nc.sync.dma_start(out=wsb[0][:, 0:NCH], in_=wt[0:P, 0:NCH])
            nc.scalar.dma_start(out=xm0[:, P:1024], in_=xh[0:P, P:1024])
            nc.sync.dma_start(out=wsb[0][:, NCH:1024], in_=wt[0:P, NCH:1024])
            nc.sync.dma_start(out=wsb[0][:, 1024:NOUT], in_=wt[0:P, 1024:NOUT])
            nc.scalar.dma_start(out=xm0[:, 1024:KIN], in_=xh[0:P, 1024:KIN])
            for k in range(1, KT // 2):
                nc.sync.dma_start(
                    out=wsb[k][:, :], in_=wt[k * P:(k + 1) * P, :]
                )

            # prioritize W k0..7 on the DMA rings: the rings round-robin all
            # queued rows, so issuing all 16 tiles at once makes early
            # k-tiles complete as late as the last ones and stalls the
            # in-order PE queue. This tiny SBUF->DRAM dma stalls the sync
            # queue until k7 lands, so k8..15 only hit the rings afterwards
            # (consumption of k8..15 starts ~6us later than that).
            wh_gate = dpool.tile([1, 1], bf16, name="wh_gate")
            nc.sync.dma_start(out=wh_gate[:, :], in_=wsb[KT // 2 - 1][0:1, 0:1])
            for k in range(KT // 2, KT):
                nc.sync.dma_start(
                    out=wsb[k][:, :], in_=wt[k * P:(k + 1) * P, :]
                )

            # PE pre-warm on the first x slice (lands ~1.5us in): keeps the
            # PE busy so the HAM activity clock-gate opens before real work;
            # results land in a psum bank that the real stream later resets
            pswarm = ppool.tile([P, NCH], f32, name="ps0", tag="ps0")
            for _ in range(16):
                nc.tensor.matmul(
                    pswarm[:, 0:P], lhsT=xm0[:, 0:P], rhs=xm0[:, 0:P],
                    start=True, stop=True,
                )

            def load_xm(m, eng):
                t = xpool.tile([P, KIN], bf16, name="xm", tag="xm")
                eng.dma_start(out=t[:, :], in_=xh[m * P:(m + 1) * P, :])
                return t

            xm_pre = [xm0, load_xm(1, nc.scalar)]

            # rendezvous the 8 cores while the weight DMAs stream in, so the
            # tail collective doesn't pay launch-skew latency
            sync_in = dpool.tile([1, 1], f32, name="sync_in")
            sync_out = dpool.tile(
                [1, NCORES], f32, name="sync_out", addr_space="Shared"
            )
            nc.gpsimd.collective_compute(
                "AllGather",
                ALU.bypass,
                replica_groups=[list(range(NCORES))],
                ins=[sync_in.opt()],
                outs=[sync_out.opt()],
            )
            # v replica loads on scalar with the x traffic (needed ~45us in)
            vsb = wpool.tile([P, NOUT], f32, name="vsb")
            nc.scalar.dma_start(out=vsb[:, :], in_=vr[:, :])

            # gate the early x prefetches behind W completion: this copy
            # stalls the gpsimd queue until the last W tile lands, so the
            # prefetch DMAs it issues next can't steal ring bandwidth from
            # the W stream (their deadline is ~48us+)
            wgate = spool.tile([1, 1], bf16, name="wgate")
            nc.gpsimd.tensor_copy(wgate[0:1, 0:1], wsb[KT - 1][0:1, 0:1])

            scores = spool.tile([P, MT], f32, name="scores")
            expv = spool.tile([P, MT], f32, name="expv")
            zrow = spool.tile([P, 1], f32, name="zrow")

            def alloc_work(m):
                psums = []
                for j in range(NCK):
                    ps = ppool.tile([P, NCH], f32, name=f"ps{j}", tag=f"ps{j}")
                    psums.append(ps)
                tmt = tpool.tile([P, NOUT], f32, name="tmt", tag="tmt")
                umt = tpool.tile([P, NOUT], f32, name="umt", tag="umt")
                acc = tpool.tile([P, NCK], f32, name="acc", tag="acc")
                return psums, tmt, umt, acc

            def drain(m, psums, tmt, umt, acc, j):
                sl = slice(j * NCH, (j + 1) * NCH)
                nc.scalar.activation(tmt[:, sl], psums[j][:, :], AF.Tanh)
                # one DVE op: umt = tanh*v, acc[:,j] = row-sum(umt)
                nc.vector.scalar_tensor_tensor(
                    out=umt[:, sl],
                    in0=tmt[:, sl],
                    scalar=1.0,
                    in1=vsb[:, sl],
                    op0=ALU.mult,
                    op1=ALU.mult,
                    accum_out=acc[:, j:j + 1],
                )

            def finish_scores(m, acc):
                nc.vector.tensor_reduce(
                    scores[:, m:m + 1], acc[:, :], AX.X, ALU.add
                )

            # tiles 0 and 1 interleaved over k: 8 matmuls per arriving W
            # k-tile keep the PE saturated while W streams in (8.4 MB takes
            # ~25us; a single tile only holds 17us of work)
            work01 = [alloc_work(0), alloc_work(1)]
            for k in range(KT):
                for m in (0, 1):
                    for j in range(NCK):
                        nc.tensor.matmul(
                            work01[m][0][j][:, :],
                            lhsT=xm_pre[m][:, k * P:(k + 1) * P],
                            rhs=wsb[k][:, j * NCH:(j + 1) * NCH],
                            start=(k == 0),
                            stop=(k == KT - 1),
                        )
            for m in (0, 1):
                psums, tmt, umt, acc = work01[m]
                for j in range(NCK):
                    drain(m, psums, tmt, umt, acc, j)
                finish_scores(m, acc)

            for m in range(2, MT):
                # early prefetches go on the gpsimd queue behind the W-gate
                # copy; later ones are gated by xpool instance reuse anyway
                if m < 10:
                    eng = nc.gpsimd
                else:
                    eng = nc.sync if m % 2 == 0 else nc.gpsimd
                xm = load_xm(m, eng)
                psums, tmt, umt, acc = alloc_work(m)

                if m < MT - 1:
                    # k-outer: the 4 matmuls per k share the stationary x
                    # tile (LDWEIGHTS dedup below)
                    for k in range(KT):
                        for j in range(NCK):
                            nc.tensor.matmul(
                                psums[j][:, :],
                                lhsT=xm[:, k * P:(k + 1) * P],
                                rhs=wsb[k][:, j * NCH:(j + 1) * NCH],
                                start=(k == 0),
                                stop=(k == KT - 1),
                            )
                    for j in range(NCK):
                        drain(m, psums, tmt, umt, acc, j)
                    finish_scores(m, acc)
                else:
                    # last tile chunk-major so each chunk drains while the
                    # next chunk's matmuls run, and in half-chunks so the
                    # final tanh+mul on the critical chain is half as long:
                    # shortens every core's path to the collective trigger
                    acc8 = tpool.tile(
                        [P, 2 * NCK], f32, name="acc8", tag="acc8"
                    )
                    NH = NCH // 2
                    for j in range(NCK):
                        for k in range(KT):
                            nc.tensor.matmul(
                                psums[j][:, :],
                                lhsT=xm[:, k * P:(k + 1) * P],
                                rhs=wsb[k][:, j * NCH:(j + 1) * NCH],
                                start=(k == 0),
                                stop=(k == KT - 1),
                            )
                        for h in range(2):
                            sl = slice(
                                j * NCH + h * NH, j * NCH + (h + 1) * NH
                            )
                            psl = slice(h * NH, (h + 1) * NH)
                            nc.scalar.activation(
                                tmt[:, sl], psums[j][:, psl], AF.Tanh
                            )
                            nc.vector.scalar_tensor_tensor(
                                out=umt[:, sl],
                                in0=tmt[:, sl],
                                scalar=1.0,
                                in1=vsb[:, sl],
                                op0=ALU.mult,
                                op1=ALU.mult,
                                accum_out=acc8[:, 2 * j + h:2 * j + h + 1],
                            )
                    nc.vector.tensor_reduce(
                        scores[:, m:m + 1], acc8[:, :], AX.X, ALU.add
                    )

            # ---- softmax over the global N via one AllReduce ----
            nc.scalar.activation(
                expv[:, :], scores[:, :], AF.Exp, accum_out=zrow[:, 0:1]
            )
            zloc = spool.tile([1, 1], f32, name="zloc")
            nc.gpsimd.tensor_reduce(
                zloc[0:1, 0:1], zrow[:, 0:1], AX.C, ALU.add
            )
            # AllGather instead of AllReduce: the gather is a 2-phase mesh
            # op vs AllReduce's 4 phases (~2-3us cheaper after the last
            # contribution); the 8-way add costs one ~80ns DVE op instead
            zin = dpool.tile([1, 1], f32, name="zin")
            zout = dpool.tile(
                [1, NCORES], f32, name="zout", addr_space="Shared"
            )
            nc.gpsimd.dma_start(out=zin[:, :], in_=zloc[0:1, 0:1])
            nc.gpsimd.collective_compute(
                "AllGather",
                ALU.bypass,
                replica_groups=[list(range(NCORES))],
                ins=[zin.opt()],
                outs=[zout.opt()],
            )
            # broadcast the 8 partials to every partition (stride-0 DRAM
            # read), reduce + reciprocal once, then scale the shard, store
            zgb = spool.tile([P, NCORES], f32, name="zgb")
            zout_bc = bass.AP(
                zout.tensor, zout.offset, [(0, P), (1, NCORES)]
            )
            nc.gpsimd.dma_start(out=zgb[:, :], in_=zout_bc)
            zp = spool.tile([P, 1], f32, name="zp")
            nc.vector.tensor_reduce(zp[:, 0:1], zgb[:, :], AX.X, ALU.add)
            rzb = spool.tile([P, 1], f32, name="rzb")
            nc.vector.reciprocal(rzb[:, 0:1], zp[:, 0:1])
            outsb = spool.tile([P, MT], f32, name="outsb")
            nc.vector.tensor_scalar_mul(outsb[:, :], expv[:, :], rzb[:, 0:1])
            nc.sync.dma_start(out=out_ext[:, :], in_=outsb[:, :])

    # run_bass_via_pjrt binds the exec primitive directly and skips the
    # finalize that bass_jit flows do; Bacc register allocation runs here.
    nc.finalize()
    _strip_redundant_ldweights(nc)
    return nc


def _strip_redundant_ldweights(nc):
    """Bacc's move_matmul_waits_to_ldweights emits one InstLdweights per
    matmul even when consecutive matmuls share the stationary operand.
    The PE keeps the loaded weights across matmuls, so an Ldweights whose
    weights AP equals the previous one's and that carries no semaphore
    waits/updates is pure redundant load time (~110ns each on the PE
    critical path). Drop them; only the matmuls (ldweights=false) remain."""
    def sig(arg):
        return (
            getattr(arg, "memref", None),
            getattr(arg, "offset", None),
            str(getattr(arg, "ap", None)),
        )

    removed = 0
    for bb in nc.main_func.blocks:
        keep = []
        last = None
        for inst in bb.instructions:
            if "Ldweights" in type(inst).__name__:
                s = sig(inst.ins[0])
                si = inst.sync_info
                if s == last and (
                    si is None or (not si.on_wait and not si.on_update)
                ):
                    removed += 1
                    continue
                last = s
            keep.append(inst)
        bb.instructions = keep
    return removed


def _prep_core_inputs(s, h, W, v):
    """Host-side layout prep: per-core tiled x^T, shared W^T, replicated v."""
    bf16 = ml_dtypes.bfloat16
    wt = np.ascontiguousarray(W.T).astype(bf16)          # [KIN, NOUT]
    vrep = np.ascontiguousarray(
        np.broadcast_to(v.reshape(1, NOUT), (P, NOUT))
    ).astype(np.float32)

    in_maps = []
    for c in range(NCORES):
        sl = slice(c * NS, (c + 1) * NS)
        x = np.concatenate([s[sl], h[sl]], axis=1)       # [NS, KIN]
        # xh[m*128+kk, k*128+rr] = x[m*128+rr, k*128+kk]
        xh = (
            x.reshape(MT, P, KT, P)
            .transpose(0, 3, 2, 1)
            .reshape(NS, KIN)
        )
        xh = np.ascontiguousarray(xh).astype(bf16)
        in_maps.append({"xh": xh, "wt": wt, "vr": vrep})
    return in_maps


_RUN_KW = {}  # test.py can inject trace=True etc.
LAST_RESULT = None


def kernel(s, h, W, v):
    from concourse.bass_utils import run_bass_kernel_spmd

    global LAST_RESULT
    s = np.asarray(s, dtype=np.float32)
    h = np.asarray(h, dtype=np.float32)
    W = np.asarray(W, dtype=np.float32)
    v = np.asarray(v, dtype=np.float32)

    in_maps = _prep_core_inputs(s, h, W, v)
    res = None
    for attempt in range(3):
        nc = _build_nc()
        try:
            res = run_bass_kernel_spmd(
                nc, in_maps, core_ids=list(range(NCORES)), **_RUN_KW
            )
            break
        except Exception:
            # transient NRT_EXEC_UNIT_UNRECOVERABLE states clear on the
            # next attempt; rebuild and retry
            if attempt == 2:
                raise
            import time
            time.sleep(15)
    LAST_RESULT = res

    outs = []
    for c in range(NCORES):
        oc = np.asarray(res.results[c]["out"], dtype=np.float32)  # [P, MT]
        outs.append(oc.T.reshape(-1))                              # rows m*128+p
    return np.concatenate(outs).reshape(1, N).astype(np.float32)
